# revision 41
# baseline (speedup 1.0000x reference)
"""GCN (7-layer, PyG GCNConv-style) on 8 Trainium2 NeuronCores.

v2 design (gather-throughput optimized):
- Nodes destination-sharded: core k owns nodes [k*12500, (k+1)*12500).
  Within a core, nodes are placed on a [128 partitions x 98 blocks] grid in
  DESCENDING IN-DEGREE order (rank r -> (p=r%128, c=r//128)), so each block's
  128 nodes have nearly equal degree and the per-block slot count
  D_b = max in-block degree wastes <5% slots.
- Per layer: stage A computes htil = dinv*(H @ W) per block (TensorE), casts
  to fp16 and AllGathers a [100352, fo_pad] fp16 table (rows in placement
  order, fo padded so that groups of G=4 rows form a 256B/512B element).
- Aggregation: per destination slot (p, slot j) one int16 GROUP index
  (= src_row//4, < 25088 so it fits dma_gather's int16 limit). Gathers run as
  896-index dma_gather instructions round-robined over 4 SWDGE queues
  (~2.4 ns/row sustained vs ~8.5 single-queue). The gathered [128, 7, G*fo]
  fp16 messages are scaled in-place by host-built masked weights
  wqg[p, slot, g] = w_e * dinv_dst * (g == src%4), then block-pieces are
  reduced over (slot, g) on the VectorE into fp32 accumulators.
- Self loops fold algebraically: out = relu(acc + dinv^2*(H@W) + b).
- Layer 7 commutes W7 past the aggregation (out = (A_hat h6) @ W7 + b7) so
  its table is dinv*h6 (10 wide) instead of a 1-wide table.
- Degrees, dinv and all floating-point math run on device; host work is
  index/layout preparation (sharding, degree sort, slot packing, fp16 cast).
"""
import sys

sys.path.insert(0, "/opt/trn_rl_repo")

from contextlib import ExitStack

import numpy as np

NC = 8
N_NODES = 100000
NLOC = N_NODES // NC            # 12500
NBLK = (NLOC + 127) // 128      # 98
NLOCP = NBLK * 128              # 12544
NTAB = NC * NLOCP               # 100352 table rows
G = 4                           # nodes per gather element
NGRP = NTAB // G                # 25088 (< 32768, int16-safe)
DIMS = [(128, 50), (50, 50), (50, 30), (30, 30), (30, 10), (10, 10), (10, 1)]
NLAYER = len(DIMS)
# gather-table padded widths per layer (G * FOP * 2B must be % 256)
FOP = [64, 64, 32, 32, 32, 32, 32]   # layer-7 table holds dinv*h6 (10 wide)
FOT = [50, 50, 30, 30, 10, 10, 10]   # real table width per layer
CHUNK_COLS = 7                  # 896 idxs per dma_gather (ring limit 64 desc/engine)


def _host_prep(x, edge_index, edge_weight):
    row = np.asarray(edge_index[0], dtype=np.int64)   # src
    col = np.asarray(edge_index[1], dtype=np.int64)   # dst
    w = np.asarray(edge_weight, dtype=np.float32)

    deg = np.bincount(col, minlength=N_NODES)

    # per-core degree-sorted placement: rank within core -> (p, c)
    place_p = np.zeros(N_NODES, np.int64)
    place_c = np.zeros(N_NODES, np.int64)
    perms = []
    for k in range(NC):
        lo, hi = k * NLOC, (k + 1) * NLOC
        order = np.argsort(-deg[lo:hi], kind="stable")  # local ids sorted by deg desc
        rank = np.empty(NLOC, np.int64)
        rank[order] = np.arange(NLOC)
        place_p[lo:hi] = rank % 128
        place_c[lo:hi] = rank // 128
        perms.append(order)
    flat = (col // NLOC) * 0  # placeholder
    # flat table row of every node: k*NLOCP + c*128 + p
    node_row = (np.arange(N_NODES) // NLOC) * NLOCP + place_c * 128 + place_p

    core_of = col // NLOC
    in_maps = []
    nslot_list = []
    per_core = []
    for k in range(NC):
        m = core_of == k
        r_k = row[m]
        c_k = col[m]
        w_k = w[m]
        p_d = place_p[c_k]
        c_d = place_c[c_k]
        # D_b per block
        blk_deg = np.zeros((128, NBLK), np.int64)
        np.add.at(blk_deg, (p_d, c_d), 1)
        D_b = blk_deg.max(axis=0)
        D_b = np.maximum(D_b, 1)
        colbase = np.zeros(NBLK + 1, np.int64)
        colbase[1:] = np.cumsum(D_b)
        nslot = int(colbase[-1])
        nslot_list.append(nslot)
        per_core.append((r_k, w_k, p_d, c_d, D_b, colbase, blk_deg))

    # pad all cores to a common slot count (multiple of CHUNK_COLS)
    NSLOT = max(nslot_list)
    NSLOT = ((NSLOT + CHUNK_COLS - 1) // CHUNK_COLS) * CHUNK_COLS

    for k in range(NC):
        r_k, w_k, p_d, c_d, D_b, colbase, blk_deg = per_core[k]
        # slot position within (p, c): running count per dest
        # order edges by (c_d, p_d) stable to assign j
        key = c_d * 128 + p_d
        order = np.argsort(key, kind="stable")
        r_s, w_s, p_s, c_s = r_k[order], w_k[order], p_d[order], c_d[order]
        ks = key[order]
        first = np.zeros(128 * NBLK + 1, np.int64)
        first[1:] = np.cumsum(np.bincount(ks, minlength=128 * NBLK))
        j = np.arange(len(ks), dtype=np.int64) - first[ks]
        slotcol = colbase[c_s] + j

        g_idx = np.zeros((128, NSLOT), np.int16)
        wqg = np.zeros((128, NSLOT, G), np.float16)
        src_row = node_row[r_s]
        g_idx[p_s, slotcol] = (src_row // G).astype(np.int16)
        wqg[p_s, slotcol, src_row % G] = w_s.astype(np.float16)

        # wrapped idx layout for dma_gather: idx i lives at (partition i%16,
        # free i//16), replicated on all 8 16-partition groups. i = col*128+p.
        arr = g_idx.T.reshape(-1)  # i = col*128 + p
        wrapped = arr.reshape(-1, 16).T  # [16, NSLOT*8]
        idx_w = np.tile(wrapped, (8, 1))

        # block -> slot-column ranges and chunk-piece decomposition
        xk = np.zeros((128, NBLK, 128), np.float32)
        lo = k * NLOC
        xloc = np.asarray(x[lo : lo + NLOC], dtype=np.float32)
        xk[place_p[lo : lo + NLOC], place_c[lo : lo + NLOC], :] = xloc

        in_maps.append(
            {
                "idx_w": idx_w.astype(np.int16),
                "wqg": wqg.reshape(128, NSLOT * G),
                "x_p": xk.reshape(128, NBLK * 128),
            }
        )

    colbases = [pc[5] for pc in per_core]
    unperms = []
    for k in range(NC):
        lo = k * NLOC
        # node local id -> (p, c); output tile is [128, NBLK] with value at (p, c)
        unperms.append((place_p[lo : lo + NLOC], place_c[lo : lo + NLOC]))
    return in_maps, NSLOT, colbases, unperms


def _build_program(NSLOT, colbase_list):
    from concourse import bass, bacc, mybir, tile

    # all cores share one program; use the max column structure. colbase_list
    # entries differ per core only through D_b; we use core 0's? No: program is
    # SPMD-identical, so slot->block mapping must be identical across cores.
    # Host pads every core to the same NSLOT; block boundaries differ per core,
    # which the program cannot express. Instead reduce per fixed column chunks
    # and add into per-block accumulators via a host-common structure:
    # we require a COMMON colbase across cores (host uses max D_b per block).
    colbase = colbase_list  # common [NBLK+1]
    NCHUNK = NSLOT // CHUNK_COLS

    nc = bacc.Bacc(
        "TRN2",
        target_bir_lowering=False,
        debug=False,
        num_devices=NC,
        num_swdge_queues=4,
    )

    f16, f32 = mybir.dt.float16, mybir.dt.float32
    x_p = nc.declare_dram_parameter("x_p", [128, NBLK * 128], mybir.dt.bfloat16, isOutput=False)
    idx_w = nc.declare_dram_parameter("idx_w", [128, NSLOT * 8], mybir.dt.int16, isOutput=False)
    wqg_d = nc.declare_dram_parameter("wqg", [128, NSLOT * G], f16, isOutput=False)
    ident = nc.declare_dram_parameter("ident", [128, 128], f32, isOutput=False)
    Ws, Bs = [], []
    for i, (fi, fo) in enumerate(DIMS):
        Ws.append(nc.declare_dram_parameter(f"W{i+1}", [fi, fo], f32, isOutput=False))
        Bs.append(nc.declare_dram_parameter(f"b{i+1}", [128, fo], f32, isOutput=False))
    w7row = nc.declare_dram_parameter("w7row", [128, 10], f32, isOutput=False)
    out_ext = nc.declare_dram_parameter("out", [128, NBLK], f32, isOutput=True)

    bounces = [nc.dram_tensor(f"bounce{i}", [NLOCP, FOP[i]], f16) for i in range(NLAYER)]
    tables = [
        nc.dram_tensor(f"table{i}", [NTAB, FOP[i]], f16, addr_space="Shared")
        for i in range(NLAYER)
    ]

    with tile.TileContext(nc) as tc, ExitStack() as ctx:
        const = ctx.enter_context(tc.tile_pool(name="const", bufs=1))
        work = ctx.enter_context(tc.tile_pool(name="work", bufs=4))
        hpool = ctx.enter_context(tc.tile_pool(name="hpool", bufs=2))
        tpool = ctx.enter_context(tc.tile_pool(name="tpool", bufs=1))
        t2pool = ctx.enter_context(tc.tile_pool(name="t2pool", bufs=2))
        msgp = ctx.enter_context(tc.tile_pool(name="msgp", bufs=12))
        accp = ctx.enter_context(tc.tile_pool(name="accp", bufs=1))
        psT = ctx.enter_context(tc.tile_pool(name="psT", bufs=4, space="PSUM"))
        psH = ctx.enter_context(tc.tile_pool(name="psH", bufs=4, space="PSUM"))

        ident_t = const.tile([128, 128], f32)
        nc.sync.dma_start(out=ident_t[:], in_=ident[:])
        bf16 = mybir.dt.bfloat16
        ident_b = const.tile([128, 128], bf16)
        nc.vector.tensor_copy(out=ident_b[:], in_=ident_t[:])
        w7row_t = const.tile([128, 10], f32)
        nc.sync.dma_start(out=w7row_t[:], in_=w7row[:])
        idx_t = const.tile([128, NSLOT * 8], mybir.dt.int16)
        nc.sync.dma_start(out=idx_t[:], in_=idx_w[:])
        wqg_t = const.tile([128, NSLOT, G], f16)
        nc.sync.dma_start(out=wqg_t[:], in_=wqg_d[:].rearrange("p (s g) -> p s g", g=G))
        W_ts, B_ts, Wb_ts = [], [], []
        for i, (fi, fo) in enumerate(DIMS):
            W_t = const.tile([fi, fo], f32, tag=f"W{i}")
            nc.sync.dma_start(out=W_t[:], in_=Ws[i][:])
            W_b = const.tile([fi, fo], bf16, tag=f"Wb{i}")
            nc.vector.tensor_copy(out=W_b[:], in_=W_t[:])
            B_t = const.tile([128, fo], f32, tag=f"B{i}")
            nc.sync.dma_start(out=B_t[:], in_=Bs[i][:])
            W_ts.append(W_t)
            B_ts.append(B_t)
            Wb_ts.append(W_b)

        # ---- degrees -> dinv, dinv2; fold dinv into wqg ----
        deg_t = const.tile([128, NBLK], f32)
        for b in range(NBLK):
            s0, s1 = int(colbase[b]), int(colbase[b + 1])
            nc.vector.tensor_reduce(
                deg_t[:, b : b + 1],
                wqg_t[:, s0:s1, :].rearrange("p s g -> p (s g)"),
                mybir.AxisListType.X,
                mybir.AluOpType.add,
            )
        sqrt_t = const.tile([128, NBLK], f32)
        nc.scalar.activation(
            out=sqrt_t[:], in_=deg_t[:], func=mybir.ActivationFunctionType.Sqrt, bias=1.0, scale=1.0
        )
        dinv_t = const.tile([128, NBLK], f32)
        nc.vector.reciprocal(out=dinv_t[:], in_=sqrt_t[:])
        dinv2_t = const.tile([128, NBLK], f32)
        nc.vector.tensor_tensor(out=dinv2_t[:], in0=dinv_t[:], in1=dinv_t[:], op=mybir.AluOpType.mult)
        for b in range(NBLK):
            s0, s1 = int(colbase[b]), int(colbase[b + 1])
            nc.vector.tensor_scalar_mul(
                wqg_t[:, s0:s1, :].rearrange("p s g -> p (s g)"),
                wqg_t[:, s0:s1, :].rearrange("p s g -> p (s g)"),
                dinv_t[:, b : b + 1],
            )

        h_cur = None
        for li, (fi, fo) in enumerate(DIMS):
            last = li == NLAYER - 1
            fop, fot = FOP[li], FOT[li]
            # ---- stage A ----
            htil = tpool.tile([128, NBLK, fop], f16, tag="htil")
            if fot < fop:
                nc.vector.memset(htil[:], 0.0)
            htil2 = t2pool.tile([128, NBLK, fo], f32, tag="htil2")
            if not last and li > 0:
                h_bf = h_bf_cur
            for b in range(NBLK):
                if last:
                    break
                if li == 0:
                    h_chunk = work.tile([128, fi], bf16, tag="xchunk")
                    nc.sync.dma_start(
                        out=h_chunk[:],
                        in_=x_p[:].rearrange("p (c f) -> p c f", f=128)[:, b, :],
                    )
                    src_ap = h_chunk[:]
                else:
                    src_ap = h_bf[:, b, :]
                pT = psT.tile([fi, 128], bf16, space="PSUM", tag="pT")
                nc.tensor.transpose(out=pT[:], in_=src_ap, identity=ident_b[:])
                hT = work.tile([fi, 128], bf16, tag="hT")
                nc.vector.tensor_copy(out=hT[:], in_=pT[:])
                pH = psH.tile([128, fo], f32, space="PSUM", tag="pH")
                nc.tensor.matmul(out=pH[:], lhsT=hT[:], rhs=Wb_ts[li][:], start=True, stop=True)
                # table row: dinv * (H@W), fp16
                nc.scalar.activation(
                    out=htil[:, b, :fot],
                    in_=pH[:],
                    func=mybir.ActivationFunctionType.Copy,
                    scale=dinv_t[:, b : b + 1],
                )
                # epilogue bias: dinv^2*(H@W) + b
                nc.vector.scalar_tensor_tensor(
                    out=htil2[:, b, :],
                    in0=pH[:],
                    scalar=dinv2_t[:, b : b + 1],
                    in1=B_ts[li][:],
                    op0=mybir.AluOpType.mult,
                    op1=mybir.AluOpType.add,
                )
            if last:
                # layer-7 commute: table = dinv * h6 (h_cur), 10 wide
                for b in range(NBLK):
                    nc.scalar.activation(
                        out=htil[:, b, :fot],
                        in_=h_cur[:, b, :],
                        func=mybir.ActivationFunctionType.Copy,
                        scale=dinv_t[:, b : b + 1],
                    )
            # ---- bounce + AllGather ----
            nc.sync.dma_start(
                out=bounces[li][:].rearrange("(c p) f -> p c f", p=128),
                in_=htil[:],
            )
            nc.gpsimd.collective_compute(
                "AllGather",
                mybir.AluOpType.bypass,
                ins=[bounces[li][:]],
                outs=[tables[li][:]],
                replica_groups=[list(range(NC))],
            )
            # ---- gather + weighted reduce ----
            half = False
            fa = fot if last else fo
            acc = accp.tile([128, NBLK, fa], f32, tag="acc")
            tbl_view = (
                tables[li][:]
                if half
                else tables[li][:].rearrange("(a b) f -> a (b f)", b=G)
            )
            elem = G * fop  # fp16 elements per gathered group
            started = [False] * NBLK
            for ci in range(NSLOT // CHUNK_COLS):
                c0 = ci * CHUNK_COLS
                msg = msgp.tile([128, CHUNK_COLS, G, fop], f16, tag="msg")
                nc.gpsimd.dma_gather(
                    msg[:].rearrange("p s g f -> p s (g f)"),
                    tbl_view,
                    idx_t[:, c0 * 8 : (c0 + CHUNK_COLS) * 8],
                    CHUNK_COLS * 128,
                    CHUNK_COLS * 128,
                    elem,
                    queue_num=ci % 4,
                )
                # scale+mask in place
                if half:
                    # msg holds [s, h, g, 32] elements; per-half mult keeps
                    # APs within the 3-free-dim ISA limit
                    msg5 = msg[:].rearrange("p s g f -> p s (g f)").rearrange(
                        "p s (h g f) -> p s h g f", h=2, g=G
                    )
                    for h in range(2):
                        nc.vector.tensor_tensor(
                            out=msg5[:, :, h, :, :],
                            in0=msg5[:, :, h, :, :],
                            in1=wqg_t[:, c0 : c0 + CHUNK_COLS, :]
                            .unsqueeze(-1)
                            .to_broadcast([128, CHUNK_COLS, G, 32]),
                            op=mybir.AluOpType.mult,
                        )
                else:
                    nc.vector.tensor_tensor(
                        out=msg[:, :, :, :fot],
                        in0=msg[:, :, :, :fot],
                        in1=wqg_t[:, c0 : c0 + CHUNK_COLS, :].unsqueeze(-1).to_broadcast(
                            [128, CHUNK_COLS, G, fot]
                        ),
                        op=mybir.AluOpType.mult,
                    )
                # reduce block pieces inside this chunk
                b_lo = int(np.searchsorted(colbase, c0, side="right")) - 1
                b_hi = int(np.searchsorted(colbase, c0 + CHUNK_COLS, side="left"))
                for b in range(b_lo, min(b_hi, NBLK)):
                    s0 = max(int(colbase[b]), c0) - c0
                    s1 = min(int(colbase[b + 1]), c0 + CHUNK_COLS) - c0
                    if s1 <= s0:
                        continue
                    if half:
                        msg5p = msg[:, s0:s1, :, :].rearrange("p s g f -> p s (g f)").rearrange(
                            "p s (h g f) -> p s h g f", h=2, g=G
                        )
                        for h in range(2):
                            view = msg5p[:, :, h, :, :].rearrange("p s g f -> p f s g")
                            dst = acc[:, b, 32 * h : 32 * h + 32]
                            if not started[b]:
                                nc.vector.tensor_reduce(
                                    dst, view, mybir.AxisListType.XY, mybir.AluOpType.add
                                )
                            else:
                                pacc = work.tile([128, 32], f32, tag="pacc")
                                nc.vector.tensor_reduce(
                                    pacc[:], view, mybir.AxisListType.XY, mybir.AluOpType.add
                                )
                                nc.vector.tensor_tensor(
                                    out=dst, in0=dst, in1=pacc[:], op=mybir.AluOpType.add
                                )
                        started[b] = True
                    else:
                        view = msg[:, s0:s1, :, :fot].rearrange("p s g f -> p f (s g)")
                        if not started[b]:
                            nc.vector.tensor_reduce(
                                acc[:, b, :fot], view, mybir.AxisListType.X, mybir.AluOpType.add
                            )
                            started[b] = True
                        else:
                            pacc = work.tile([128, fot], f32, tag="pacc")
                            nc.vector.tensor_reduce(
                                pacc[:], view, mybir.AxisListType.X, mybir.AluOpType.add
                            )
                            nc.vector.tensor_tensor(
                                out=acc[:, b, :fot], in0=acc[:, b, :fot], in1=pacc[:], op=mybir.AluOpType.add
                            )
            # ---- epilogue ----
            if not last:
                # per-block-group epilogue: finer deps let the next layer's
                # stage A overlap this layer's gather tail
                h_next = hpool.tile([128, NBLK, fo], f32, tag="h")
                h_bfn = tpool.tile([128, NBLK, fo], bf16, tag="hbf")
                for g0 in range(0, NBLK, 14):
                    g1 = min(g0 + 14, NBLK)
                    nc.vector.tensor_tensor(
                        out=acc[:, g0:g1, :fo],
                        in0=acc[:, g0:g1, :fo],
                        in1=htil2[:, g0:g1, :],
                        op=mybir.AluOpType.add,
                    )
                    nc.scalar.activation(
                        out=h_next[:, g0:g1, :],
                        in_=acc[:, g0:g1, :fo],
                        func=mybir.ActivationFunctionType.Relu,
                    )
                    # bf16 copy on ACT (idle during gathers) so the next
                    # layer's PE transposes can start under the gather tail
                    nc.scalar.activation(
                        out=h_bfn[:, g0:g1, :],
                        in_=h_next[:, g0:g1, :],
                        func=mybir.ActivationFunctionType.Copy,
                    )
                h_cur = h_next
                h_bf_cur = h_bfn
            else:
                # out = (acc + dinv*htil7) @ W7 + b7 ; htil7 = dinv*h6
                for b in range(NBLK):
                    nc.vector.scalar_tensor_tensor(
                        out=acc[:, b, :],
                        in0=htil[:, b, :fot],
                        scalar=dinv_t[:, b : b + 1],
                        in1=acc[:, b, :],
                        op0=mybir.AluOpType.mult,
                        op1=mybir.AluOpType.add,
                    )
                nc.vector.tensor_tensor(
                    out=acc[:],
                    in0=acc[:],
                    in1=w7row_t[:].unsqueeze(1).to_broadcast([128, NBLK, fot]),
                    op=mybir.AluOpType.mult,
                )
                out_sb = work.tile([128, NBLK], f32, tag="outsb")
                nc.vector.tensor_reduce(
                    out_sb[:], acc[:], mybir.AxisListType.X, mybir.AluOpType.add
                )
                nc.vector.tensor_tensor(
                    out=out_sb[:],
                    in0=out_sb[:],
                    in1=B_ts[6][:, 0:1].to_broadcast([128, NBLK]),
                    op=mybir.AluOpType.add,
                )
        nc.sync.dma_start(out=out_ext[:], in_=out_sb[:])

    nc.finalize()
    return nc


LAST_EXEC_NS = None
LAST_TRACE = None


def kernel(x, edge_index, edge_weight, W1, b1, W2, b2, W3, b3, W4, b4, W5, b5, W6, b6, W7, b7):
    import os

    from concourse.bass_utils import run_bass_kernel_spmd

    x = np.asarray(x, dtype=np.float32)
    row = np.asarray(edge_index[0], dtype=np.int64)
    col = np.asarray(edge_index[1], dtype=np.int64)
    w = np.asarray(edge_weight, dtype=np.float32)

    # --- host prep with a COMMON slot structure across cores ---
    deg = np.bincount(col, minlength=N_NODES)
    place_p = np.zeros(N_NODES, np.int64)
    place_c = np.zeros(N_NODES, np.int64)
    for k in range(NC):
        lo, hi = k * NLOC, (k + 1) * NLOC
        order = np.argsort(-deg[lo:hi], kind="stable")
        rank = np.empty(NLOC, np.int64)
        rank[order] = np.arange(NLOC)
        place_p[lo:hi] = rank % 128
        place_c[lo:hi] = rank // 128
    node_row = (np.arange(N_NODES) // NLOC) * NLOCP + place_c * 128 + place_p

    core_of = col // NLOC
    blk_deg_max = np.zeros(NBLK, np.int64)
    per_core = []
    for k in range(NC):
        m = core_of == k
        r_k, c_k, w_k = row[m], col[m], w[m]
        p_d, c_d = place_p[c_k], place_c[c_k]
        bd = np.zeros((128, NBLK), np.int64)
        np.add.at(bd, (p_d, c_d), 1)
        blk_deg_max = np.maximum(blk_deg_max, bd.max(axis=0))
        per_core.append((r_k, w_k, p_d, c_d))
    D_b = np.maximum(blk_deg_max, 1)
    colbase = np.zeros(NBLK + 1, np.int64)
    colbase[1:] = np.cumsum(D_b)
    NSLOT = int(colbase[-1])
    NSLOT = ((NSLOT + CHUNK_COLS - 1) // CHUNK_COLS) * CHUNK_COLS

    in_maps = []
    for k in range(NC):
        r_k, w_k, p_d, c_d = per_core[k]
        key = c_d * 128 + p_d
        order = np.argsort(key, kind="stable")
        r_s, w_s, p_s, c_s = r_k[order], w_k[order], p_d[order], c_d[order]
        ks = key[order]
        first = np.zeros(128 * NBLK + 1, np.int64)
        first[1:] = np.cumsum(np.bincount(ks, minlength=128 * NBLK))
        j = np.arange(len(ks), dtype=np.int64) - first[ks]
        slotcol = colbase[c_s] + j

        g_idx = np.zeros((128, NSLOT), np.int16)
        wqg = np.zeros((128, NSLOT, G), np.float16)
        src_row = node_row[r_s]
        g_idx[p_s, slotcol] = (src_row // G).astype(np.int16)
        wqg[p_s, slotcol, src_row % G] = w_s.astype(np.float16)

        arr = g_idx.T.reshape(-1)
        wrapped = arr.reshape(-1, 16).T
        idx_w = np.tile(wrapped, (8, 1)).astype(np.int16)

        lo = k * NLOC
        xk = np.zeros((128, NBLK, 128), np.float32)
        xk[place_p[lo : lo + NLOC], place_c[lo : lo + NLOC], :] = x[lo : lo + NLOC]
        import ml_dtypes
        xk_bf = xk.astype(ml_dtypes.bfloat16)

        in_maps.append(
            {
                "idx_w": idx_w,
                "wqg": wqg.reshape(128, NSLOT * G),
                "x_p": xk_bf.reshape(128, NBLK * 128),
            }
        )

    Wmats = [np.asarray(Wm, dtype=np.float32) for Wm in (W1, W2, W3, W4, W5, W6, W7)]
    bvecs = [np.tile(np.asarray(b, dtype=np.float32).reshape(1, -1), (128, 1)) for b in (b1, b2, b3, b4, b5, b6, b7)]
    ident = np.eye(128, dtype=np.float32)
    w7r = np.tile(Wmats[6][:, 0].reshape(1, -1), (128, 1)).astype(np.float32)
    for mdl in in_maps:
        for i in range(NLAYER):
            mdl[f"W{i+1}"] = Wmats[i]
            mdl[f"b{i+1}"] = bvecs[i]
        mdl["ident"] = ident
        mdl["w7row"] = w7r

    nc = _build_program(NSLOT, colbase)
    trace = os.environ.get("BASS_GCN_TRACE", "0") == "1"
    kw = {}
    if trace:
        kw = dict(trace=True, tmpdir="/tmp/gcn_trace")
        os.makedirs("/tmp/gcn_trace", exist_ok=True)
    res = run_bass_kernel_spmd(nc, in_maps, list(range(NC)), **kw)
    global LAST_EXEC_NS, LAST_TRACE
    LAST_EXEC_NS = res.exec_time_ns
    LAST_TRACE = res.instructions_and_trace[1] if res.instructions_and_trace else None

    out = np.zeros((N_NODES, 1), np.float32)
    for k in range(NC):
        pm = res.results[k]["out"]  # [128, NBLK]
        lo = k * NLOC
        out[lo : lo + NLOC, 0] = pm[place_p[lo : lo + NLOC], place_c[lo : lo + NLOC]]
    return out
